# revision 22
# baseline (speedup 1.0000x reference)
"""Multi-head attention (raw-reshape variant) on 8 trn2 NeuronCores.

Shapes: B=2, S=2048, D=1024, H=16, dh=64.  The reference uses a raw
reshape (B,S,D)->(B,H,S,dh) (NOT a head transpose), so head h only sees
projected rows [128h, 128h+128).  Each (b, h) pair is therefore fully
independent: core c handles b=c//4 and the 4 heads of seq-block c%4.
No collectives; the host shards inputs and concatenates outputs.

Folded index convention per pair (128 input rows r, 1024 dims):
  s' = 16 r + t,  dm = 64 t + d   (t in [0,16), d in [0,64))
  Qfold[s', d] = Proj[r, 64 t + d]

The kernel works in a PERMUTED query order q'' = 128 t + r (t-major)
so every on-chip slice is contiguous; the host permutes the mask
columns to match and the final projection un-permutes for free.

v2+ schedule (single dense PE stream; ACT-exp ~143us is the floor):
  prologue: K-asm then Q-asm (both t-groups per chunk arrival, so the
    PE is DMA-paced without long idles and HAM stays warm), V-proj.
    Mask tiles 0-3 load immediately on the SWDGE queue; tiles 4-15 are
    queued on the sync queue behind the asm loads so those transfers
    get HBM bandwidth first; w_o blocks are staged once into both
    partition halves of wo_all.  A dummy exp preloads the ACT table.
  pipeline (128 steps, ACT-paced ~1.15us/step): phases (p,qh) ordered
    all qh=0 then all qh=1.  Per step: S^T (2 MMs, K=64), exp on ACT
    (scale fused), mask-mul on DVE, PV drained at LAG=3.  psO is
    single-buffered; each phase's psO is copied PSUM->SBUF right after
    its last PV so the bank frees in <1 step; recip+normalize read the
    copy, and for qh=0 phases their emission is deferred into the next
    phase (t==3 drain) so the new phase's mask-muls get ahead of them
    in the DVE queue.  Out-proj blobs are emitted CONTIGUOUSLY in the
    inter-phase gaps after each pair's qh=1 normalize -- spreading a
    PSUM accumulation chain's matmuls across pipeline steps corrupts
    results on hardware (PE tile-concurrency hazard), a contiguous
    blob between PV chains is the proven-safe shape.
  tail: the last pair's out-proj blob, PSUM->SBUF copies, output DMA.

PSUM budget (8 banks): stt 2x[128,1024]=4, psO 1x[128,1024]=2,
filler pool 2x[128,512]=2.
"""

import ml_dtypes
import numpy as np

import concourse.bass as bass
import concourse.mybir as mybir
import concourse.tile as tile
from concourse import bacc
from concourse.bass_utils import run_bass_kernel_spmd

F32 = mybir.dt.float32
F16 = mybir.dt.float16
F8 = mybir.dt.float8e4

B, S, D, H, DH = 2, 2048, 1024, 16, 64
N_CORES = 8
CORE_ROWS = 512          # seq rows per core
N_PAIRS = 4              # (b, h) pairs per core
EXP_SCALE = 0.125        # 1/sqrt(dh)
LAG = 3

_NC = None


def _build_program():
    nc = bacc.Bacc()

    # host-concatenated [w_chunk | x_chunk] per contraction chunk k
    qasm = nc.dram_tensor("qasm", [8, 128, 1536], F16, kind="ExternalInput")
    kasm = nc.dram_tensor("kasm", [8, 128, 1536], F16, kind="ExternalInput")
    vasm = nc.dram_tensor("vasm", [8, 128, 1536], F16, kind="ExternalInput")
    woTblk = nc.dram_tensor("woTblk", [16, 64, D], F16, kind="ExternalInput")
    maskc_d = nc.dram_tensor("maskc", [S, S], F16, kind="ExternalInput")
    out_d = nc.dram_tensor("out", [CORE_ROWS, D], F32, kind="ExternalOutput")

    with tile.TileContext(nc) as tc:
        with tc.tile_pool(name="persist", bufs=1) as persist:
            maskc_sb = [persist.tile([128, S], F16, tag=f"mask{t}", name=f"mask{t}")
                        for t in range(16)]

            # col = 2048 g + 128 t + r  (q''-order, group-major)
            qt_all = persist.tile([128, 2 * S], F16, tag="qt", name="qt")
            kt_all = persist.tile([128, 2 * S], F16, tag="kt", name="kt")
            qt = [qt_all[:, S * g:S * (g + 1)] for g in range(2)]
            kt = [kt_all[:, S * g:S * (g + 1)] for g in range(2)]
            vaug = [persist.tile([128, 2048], F16, tag=f"vaug{p}", name=f"vaug{p}")
                    for p in range(N_PAIRS)]
            stack = [persist.tile([128, S], F16, tag=f"stack{g}", name=f"stack{g}")
                     for g in range(2)]
            outsb = [persist.tile([128, D], F32, tag=f"outsb{p}", name=f"outsb{p}")
                     for p in range(N_PAIRS)]
            # w_o blocks staged once; both partition halves hold the same
            # data so hp=0 and hp=1 out-proj can slice at their base
            wo_all = persist.tile([128, 16 * 1024], F16, tag="wo_all",
                                  name="wo_all")

            # mask tiles 0-3 early on the SWDGE queue (needed from pipeline
            # step 0); tiles 4-15 are deferred (emitted below, gated on the
            # asm DMAs) so kasm/qasm transfers get full HBM bandwidth and
            # the assembly matmuls stay PE-paced (HAM stays warm)
            for t in range(4):
                nc.gpsimd.dma_start(out=maskc_sb[t][:, :], in_=maskc_d[t::16, :])

            # per-t blocks are [ones (64) | V_t (64)]: the denominator rows
            # land at psO partitions 0:63 (reciprocal_approx_fast needs a
            # base-partition-0 input) and O^T at 64:127
            for p in range(N_PAIRS):
                va3 = vaug[p][:, :].rearrange("p (t c) -> p t c", c=128)
                nc.vector.memset(va3[:, :, 0:64], 1.0)

            # preload the exp table set (~2.7us) while the prologue runs
            warm = persist.tile([128, 16], F32, tag="warm", name="warm")
            nc.vector.memset(warm[:, :], 0.0)
            nc.scalar.activation(warm[:, :], warm[:, :],
                                 mybir.ActivationFunctionType.Exp)

            dst_q = qt_all[:, :].rearrange("p (g t r) -> p g t r", g=2, t=16)
            dst_k = kt_all[:, :].rearrange("p (g t r) -> p g t r", g=2, t=16)

            # ---------------- prologue ----------------
            # Assembly processes BOTH t-groups per chunk arrival (one DMA-
            # paced pass, 32 MMs per chunk) so the PE never idles long
            # enough for HAM to re-throttle.  Accumulators are [128,512]
            # (tgp in the free dim); the k==0/tgp==0 matmul's start=True
            # zeroes the whole bank so tgp1's k==0 uses start=False.
            with tc.tile_pool(name="pro_mt", bufs=16) as promt:
                kmts, qmts = [], []
                for k in range(8):
                    mt = promt.tile([128, 1536], F16, tag="kmt", name="kmt")
                    nc.sync.dma_start(out=mt[:, :], in_=kasm[k])
                    kmts.append(mt)
                for k in range(8):
                    mt = promt.tile([128, 1536], F16, tag="kmt", name="qmt")
                    nc.sync.dma_start(out=mt[:, :], in_=qasm[k])
                    qmts.append(mt)


                with tc.tile_pool(name="pro_ps", bufs=8, space="PSUM") as props:
                    for mts, dst4 in ((kmts, dst_k), (qmts, dst_q)):
                        ps8 = [props.tile([128, 512], F32, tag="asm", name="asmps")
                               for _ in range(8)]
                        for k in range(8):
                            x3 = mts[k][:, 1024:1536].rearrange(
                                "p (i j) -> p i j", j=128)
                            for tgp in range(2):
                                for u in range(8):
                                    t = 8 * tgp + u
                                    for par in range(2):
                                        nc.tensor.matmul(
                                            ps8[u][64 * par:64 * (par + 1),
                                                   256 * tgp:256 * (tgp + 1)],
                                            lhsT=mts[k][:, 64 * t:64 * (t + 1)],
                                            rhs=x3[:, par::2, :],
                                            start=(k == 0 and tgp == 0),
                                            stop=(k == 7 and tgp == 1),
                                            skip_group_check=True,
                                        )
                        for u in range(8):
                            for tgp in range(2):
                                t = 8 * tgp + u
                                src2 = ps8[u][:, 256 * tgp:256 * (tgp + 1)] \
                                    .rearrange("p (g c) -> p g c", g=2)
                                nc.vector.tensor_copy(dst4[:, :, t, :], src2)
                        if dst4 is dst_q:
                            # bridge the scatter pile: the V-proj psum pool
                            # can't open until all 16 Q-scatters drain on
                            # the DVE (~7us); these warm singles run as
                            # soon as slot 0's scatter completes, so the
                            # PE never idles past HAM's window here
                            pj = props.tile([128, 512], F32, tag="asm",
                                            name="projunk")
                            for _ in range(24):
                                nc.tensor.matmul(
                                    pj[:, :],
                                    lhsT=qmts[0][:, 0:128],
                                    rhs=qmts[0][:, 0:512],
                                    start=True, stop=True,
                                )

                # V projection: p-outer / k-inner, scattering each pair
                # right after its chain so the last DVE scatter finishes
                # ~with the last matmul -- the attention pools' boundary
                # wait then stays under HAM's idle window and the pipeline
                # enters warm.  vasm chunks rotate into freed kmt slots.
                with tc.tile_pool(name="v_ps", bufs=4, space="PSUM") as vpsum:
                    vmts = []
                    for k in range(8):
                        mt = promt.tile([128, 1536], F16, tag="kmt", name="vmt")
                        nc.sync.dma_start(out=mt[:, :], in_=vasm[k])
                        vmts.append(mt)
                    # deferred mask tiles on the sync queue: these issues
                    # sit behind the vasm DMAs (gated on K-asm finishing
                    # with the mt slots), so kasm/qasm transfers get full
                    # HBM bandwidth first.
                    for t in range(4, 16):
                        nc.sync.dma_start(out=maskc_sb[t][:, :],
                                          in_=maskc_d[t::16, :])
                    psv = [vpsum.tile([128, 1024], F32, tag="psv", name="psv")
                           for _ in range(N_PAIRS)]
                    for p in range(N_PAIRS):
                        for k in range(8):
                            for oc in range(2):
                                nc.tensor.matmul(
                                    psv[p][:, 512 * oc:512 * (oc + 1)],
                                    lhsT=vmts[k][:, 1024 + 128 * p:1024 + 128 * (p + 1)],
                                    rhs=vmts[k][:, 512 * oc:512 * (oc + 1)],
                                    start=(k == 0), stop=(k == 7),
                                )
                        src = psv[p][:, :].rearrange("p (t c) -> p t c", c=64)
                        dst3 = vaug[p][:, :].rearrange("p (t c) -> p t c", c=128)
                        nc.vector.tensor_copy(dst3[:, :, 64:128], src)

            # stage w_o blocks into both partition halves (DMA only; lands
            # well before the first out-proj blob needs it)
            for t in range(16):
                nc.sync.dma_start(out=wo_all[0:64, 1024 * t:1024 * (t + 1)],
                                  in_=woTblk[t])
                nc.sync.dma_start(out=wo_all[64:128, 1024 * t:1024 * (t + 1)],
                                  in_=woTblk[t])

            # ---------------- attention + output ----------------
            with tc.tile_pool(name="p_pool", bufs=1) as ppool, \
                 tc.tile_pool(name="norm", bufs=1) as npool, \
                 tc.tile_pool(name="st_ps", bufs=2, space="PSUM") as stpool, \
                 tc.tile_pool(name="o_ps", bufs=1, space="PSUM") as opool, \
                 tc.tile_pool(name="f_ps", bufs=2, space="PSUM") as fpool:

                def emit_outproj(g, hp):
                    """Final projection for pair (g, hp), emitted as ONE
                    contiguous blob (chain MMs spread across pipeline steps
                    corrupt results when other accumulation chains are open;
                    a blob in the inter-phase gap is the proven-safe shape).
                    Two oc chains in parallel psum tiles; rhs slices the
                    pre-staged wo_all so the blob is pure matmul."""
                    p = 2 * g + hp
                    lo, hi = 64 * hp, 64 * (hp + 1)
                    psF = [fpool.tile([128, 512], F32, tag="fps",
                                      name=f"psF{oc}") for oc in range(2)]
                    for t in range(16):
                        for oc in range(2):
                            nc.tensor.matmul(
                                psF[oc][:, :],
                                lhsT=stack[g][lo:hi, 128 * t:128 * (t + 1)],
                                rhs=wo_all[lo:hi,
                                           1024 * t + 512 * oc:1024 * t + 512 * (oc + 1)],
                                start=(t == 0), stop=(t == 15),
                            )
                    for oc in range(2):
                        nc.vector.tensor_copy(outsb[p][:, 512 * oc:512 * (oc + 1)],
                                              psF[oc][:, :])
                    nc.sync.dma_start(out=out_d[128 * p:128 * (p + 1), :],
                                      in_=outsb[p][:, :])

                # flat pipeline: all qh=0 phases, then all qh=1
                phases = [(p, 0) for p in range(N_PAIRS)] + \
                         [(p, 1) for p in range(N_PAIRS)]
                flat = [(p, qh, t) for (p, qh) in phases for t in range(16)]
                queue = []
                psO_box = [None]
                pending_norm = []

                def normalize(osb, p, qh):
                    """recip + normalize-mul from the SBUF copy of psO.
                    recip computed at base partition 0, then DMA-shifted to
                    partitions 64:127 so every SBUF operand of the mul
                    shares base partition 64 (walrus rule)."""
                    g, hp = p // 2, p % 2
                    recip = npool.tile([128, 1024], F32, tag="recip",
                                       name="recip", bufs=2)
                    nc.vector.reciprocal_approx_fast(recip[0:64, :],
                                                     osb[0:64, :])
                    nc.sync.dma_start(out=recip[64:128, :], in_=recip[0:64, :])
                    if hp == 1:
                        nc.vector.tensor_mul(
                            stack[g][64:128, 1024 * qh:1024 * (qh + 1)],
                            osb[64:128, :], recip[64:128, :])
                    else:
                        tmpa = npool.tile([128, 1024], F16, tag="tmpa",
                                          name="tmpa", bufs=2)
                        nc.vector.tensor_mul(tmpa[64:128, :], osb[64:128, :],
                                             recip[64:128, :])
                        nc.sync.dma_start(
                            out=stack[g][0:64, 1024 * qh:1024 * (qh + 1)],
                            in_=tmpa[64:128, :])

                junk_box = [None]

                def warm_mm(n):
                    """Harmless single matmuls (same proven-safe class as
                    S^T) that keep the PE array active so HAM stays warm
                    through fill and drain phases."""
                    if junk_box[0] is None:
                        junk_box[0] = fpool.tile([128, 512], F32, tag="fps",
                                                 name="junk")
                    for _ in range(n):
                        nc.tensor.matmul(
                            junk_box[0][:, :],
                            lhsT=kt[0][0:64, 0:128],
                            rhs=qt[0][0:64, 0:512],
                            start=True, stop=True,
                        )

                def drain_one():
                    p, qh, t, pm = queue.pop(0)
                    g, hp = p // 2, p % 2
                    if t == 0:
                        psO_box[0] = opool.tile([128, 1024], F32, tag="o",
                                                name="psO")
                    psO = psO_box[0]
                    # deferred normalize: emitted a few steps into the next
                    # phase so the new phase's mask-muls get ahead of it in
                    # the DVE queue (recip/mul are not urgent; only the osb
                    # copy gates the psO bank release)
                    if t == 3 and pending_norm:
                        normalize(*pending_norm.pop(0))
                    for sc in range(2):
                        nc.tensor.matmul(
                            psO[:, 512 * sc:512 * (sc + 1)],
                            lhsT=vaug[p][:, 128 * t:128 * (t + 1)],
                            rhs=pm[:, 512 * sc:512 * (sc + 1)],
                            start=(t == 0), stop=(t == 15),
                        )
                    if t < 15:
                        return
                    # phase (p, qh) complete: free the psO bank fast via a
                    # PSUM->SBUF copy.  psO rows 0:63 hold the broadcast
                    # denominator, rows 64:127 hold O^T.
                    osb = npool.tile([128, 1024], F32, tag="osb", name="osb",
                                     bufs=2)
                    nc.vector.tensor_copy(osb[:, :], psO[:, :])
                    if qh == 1:
                        # out-proj blob due: flush all pending normalizes
                        # (including this phase's) first
                        for args in pending_norm:
                            normalize(*args)
                        pending_norm.clear()
                        normalize(osb, p, qh)
                        if p == 3:
                            # final blob: pad with warm singles so its
                            # t>=8 matmuls don't idle-wait (and cool HAM)
                            # while the normalize/recip-shift lands
                            warm_mm(12)
                        emit_outproj(g, hp)
                    else:
                        pending_norm.append((osb, p, qh))

                for p, qh, t in flat:
                    g, hp = p // 2, p % 2
                    lo, hi = 64 * hp, 64 * (hp + 1)
                    if len(queue) > LAG:
                        drain_one()
                    stt = stpool.tile([128, 1024], F32, tag="st", name="stt")
                    for sc in range(2):
                        nc.tensor.matmul(
                            stt[:, 512 * sc:512 * (sc + 1)],
                            lhsT=kt[g][lo:hi, 128 * t:128 * (t + 1)],
                            rhs=qt[g][lo:hi,
                                      1024 * qh + 512 * sc:1024 * qh + 512 * (sc + 1)],
                            start=True, stop=True,
                        )
                    praw = ppool.tile([128, 1024], F16, tag="praw", name="praw",
                                      bufs=3)
                    nc.scalar.activation(praw[:, :], stt[:, :],
                                         mybir.ActivationFunctionType.Exp,
                                         scale=EXP_SCALE)
                    pm = ppool.tile([128, 1024], F16, tag="pm", name="pm",
                                    bufs=LAG + 3)
                    nc.vector.tensor_mul(pm[:, :], praw[:, :],
                                         maskc_sb[t][:, 1024 * qh:1024 * (qh + 1)])
                    queue.append((p, qh, t, pm))
                # drain tail: keep the PE active between the last PVs
                while queue:
                    warm_mm(2)
                    drain_one()

    nc.finalize()
    return nc


def build_in_maps(inputs):
    q = np.asarray(inputs["q"], dtype=np.float32)
    k = np.asarray(inputs["k"], dtype=np.float32)
    v = np.asarray(inputs["v"], dtype=np.float32)
    mask = np.asarray(inputs["mask"])
    w_q = np.asarray(inputs["w_q"], dtype=np.float32)
    w_k = np.asarray(inputs["w_k"], dtype=np.float32)
    w_v = np.asarray(inputs["w_v"], dtype=np.float32)
    w_o = np.asarray(inputs["w_o"], dtype=np.float32)

    wqT = np.ascontiguousarray(w_q.T).astype(np.float16).reshape(8, 128, D)
    wkT = np.ascontiguousarray(w_k.T).astype(np.float16).reshape(8, 128, D)
    wvT = np.ascontiguousarray(w_v.T).astype(np.float16).reshape(8, 128, D)
    woTblk = np.ascontiguousarray(w_o.T.reshape(16, 64, D)).astype(np.float16)
    # St rows are k'; columns are q'' = 128 t + r (permuted query order):
    # maskc[k', 128 t + r] = 1 - mask[b][q' = 16 r + t, k']
    maskc = []
    for b in range(B):
        mt_ = (~mask[b]).T.astype(np.float16)          # [k', q']
        mp = mt_.reshape(S, 128, 16).transpose(0, 2, 1).reshape(S, S)
        maskc.append(np.ascontiguousarray(mp))

    in_maps = []
    for c in range(N_CORES):
        b, sb = c // 4, c % 4
        rows = slice(CORE_ROWS * sb, CORE_ROWS * (sb + 1))
        xqT = np.ascontiguousarray(q[b, rows].T).astype(np.float16).reshape(8, 128, CORE_ROWS)
        xkT = np.ascontiguousarray(k[b, rows].T).astype(np.float16).reshape(8, 128, CORE_ROWS)
        xvT = np.ascontiguousarray(v[b, rows].T).astype(np.float16).reshape(8, 128, CORE_ROWS)
        in_maps.append({
            "qasm": np.concatenate([wqT, xqT], axis=2),
            "kasm": np.concatenate([wkT, xkT], axis=2),
            "vasm": np.concatenate([wvT, xvT], axis=2),
            "woTblk": woTblk,
            "maskc": maskc[b],
        })
    return in_maps


def kernel(q, k, v, mask, w_q, w_k, w_v, w_o):
    global _NC
    if _NC is None:
        _NC = _build_program()

    in_maps = build_in_maps(dict(q=q, k=k, v=v, mask=mask,
                                 w_q=w_q, w_k=w_k, w_v=w_v, w_o=w_o))
    res = run_bass_kernel_spmd(_NC, in_maps, list(range(N_CORES))).results

    out = np.empty((B, S, D), dtype=np.float32)
    for c in range(N_CORES):
        b, sb = c // 4, c % 4
        out[b, CORE_ROWS * sb:CORE_ROWS * (sb + 1)] = res[c]["out"]
    return out
